# revision 32
# baseline (speedup 1.0000x reference)
"""BPR pairwise softplus loss on 8 Trainium2 NeuronCores.

loss = mean_b sum_{i<K, j>=K, both valid} softplus(pred[b,j] - pred[b,i])

Strategy (data parallel over batch, 32 rows/core), folding FOUR negatives
per ln via elementary symmetric polynomials:

  prod_{m=1..4} (1 + F*E_m) = 1 + F*c1 + F^2*c2 + F^3*c3 + F^4*c4
  =>  sum_m softplus(n_m - p) = ln(1 + sum_k F^k c_k),  F = exp(-p), E = exp(n)

Pack-free layout: partition = 4*r + k (row-major, power k innermost), so the
8 contraction partitions of row pair t = (2t, 2t+1) are the contiguous range
8t..8t+8 and the matmul reads the power tile P and coefficient tile q
DIRECTLY - no SBUF->SBUF pack DMAs (each DMA hop costs ~2.7us of fixed
latency: 565 SEQ + 625 HWDGE + 650 engine delay + transfer + 900 sem).

  - P_v[4r+k, 64h+p] = F^{k+1}[r, p] masked to half h == r%2 AND to pairs
    with (r//2)%4 == v: each P_v is ONE ScalarE exp with per-partition
    scale -(k+1) / bias 0 on live partitions and scale 0 / bias -100
    (exp -> 0) elsewhere; the column-interleave mask is folded into the
    host data (+50 fill -> exp(-(k+1)*50) = 0).
  - q[4r+k, j] = c_{k+1}[r, j]: pair folds a/m then masked placement with
    per-partition 0/1 scalar masks via scalar_tensor_tensor:
      q = M1*(a1+a2) + M2*(m1+m2+a1*a2) + M3*(a1*m2+a2*m1) + M4*(m1*m2)
  - 16 matmuls (contraction 32 = one aligned block of 4 pairs, lhsT P_v
    zeroing the other 3 pairs, free 112) straight into PSUM; asymmetric Ln
    passes (N0 pairs then 16-N0) with accum_out row sums; one output DMA.
    (PE tiling: operands must sit at partition base 0/32/64/96 with an
    explicit tile_position, hence the 32-block contraction.)
  - Dummy warm matmuls from ~1us keep the PE p-state ramping so the real
    matmuls run at full clock (a PE idle gap resets the 3us ramp).

Invalid slots (target == -1) fold into the prediction on the host: invalid
positives -> +50 (F^k -> 0), invalid negatives -> -50 (E -> 0).
"""
import sys

sys.path.insert(0, "/opt/trn_rl_repo")

import numpy as np
import ml_dtypes

import concourse.bass as bass
import concourse.mybir as mybir
from concourse import bacc
import concourse.hw_specs as hw_specs
from concourse.tile import TileContext
from concourse.bass_utils import run_bass_kernel_spmd

B, N, K = 256, 512, 64
NC = 8
RPC = B // NC            # 32 batch rows per core
NPAIR = RPC // 2         # 16 row pairs (2t, 2t+1)
NEG = N - K              # 448 negatives per row
G = 4                    # negatives folded per ln
NQ = NEG // G            # 112 quad groups per row
NPASS = 2                # Ln passes
N0 = 4                   # row pairs in Ln pass 0 (rest in pass 1); multiple
                         # of 4 so each pass covers whole PSUM banks
N_WARM = 336             # PE clock warm-up dummy matmuls
N_WARM2 = 21             # filler matmuls between the last wave and the q
                         # matmuls so PE never idles (idle resets p-state)
N_FILL = (38, 29)        # fillers after waves 1 and 2
WARM_COLS = 16           # free size of each warm matmul

_PROG_CACHE = {}

EXP = mybir.ActivationFunctionType.Exp
LN = mybir.ActivationFunctionType.Ln
F32 = mybir.dt.float32
BF16 = mybir.dt.bfloat16


def _patch_act_tables():
    """Make natural_log_exp_and_others the only table set advertising exp/ln
    so Bacc's table-load pass emits a single ACT_TABLE_LOAD."""
    if getattr(hw_specs.get_activation_tables, "_bpr_patched", False):
        return
    orig_fn = hw_specs.get_activation_tables

    def patched(arch):
        d = orig_fn(arch)
        out = {}
        for name, funcs in d.items():
            if name != "natural_log_exp_and_others" and (EXP in funcs
                                                         or LN in funcs):
                funcs = funcs - {EXP, LN}
            out[name] = funcs
        return out

    patched._bpr_patched = True
    hw_specs.get_activation_tables = patched
    bacc.get_activation_tables = patched

    # Bass.__init__ materializes 4 const APs via gpsimd.memset, serializing
    # ~440ns on Pool before the start barrier that gates the first input
    # DMA.  Spread them DVE/Pool round-robin to halve that chain.
    orig_memset = bass.BassEitherVectorEngine.memset

    def memset_rr(self, ap, constant):
        name = getattr(getattr(ap, "tensor", None), "name", "") or ""
        if name in ("const-bfloat16-1.0", "const-uint8-127"):
            return None  # dead consts (no readers in this program)
        if name.startswith("const-"):
            b = self.bass
            rr = getattr(b, "_bpr_const_rr", 0)
            b._bpr_const_rr = rr + 1
            eng = b.vector if rr % 2 == 0 else b.gpsimd
            if eng is not self:
                return orig_memset(eng, ap, constant)
        return orig_memset(self, ap, constant)

    bass.BassEitherVectorEngine.memset = memset_rr

    # Skip the all-engine barrier Bass.__init__ emits right after the const
    # memsets — it serializes the first input DMA behind them, but nothing
    # the DMA queue touches depends on the consts, and the first const
    # consumer (an ACT op at ~1.7us) runs >1.5us after the memsets land.
    orig_barrier = bass.Bass.all_engine_barrier

    def barrier_skip_init(self, *, sem_only=False):
        if not hasattr(self, "_init_sbuf_base"):
            return None  # the one call during __init__
        return orig_barrier(self, sem_only=sem_only)

    bass.Bass.all_engine_barrier = barrier_skip_init


def build_program(nreps: int = 1):
    if nreps in _PROG_CACHE:
        return _PROG_CACHE[nreps]
    _patch_act_tables()
    nc = bacc.Bacc("TRN2", target_bir_lowering=False, debug=False,
                   num_devices=NC)
    negs_d = nc.dram_tensor("negs", [RPC, NEG], BF16, kind="ExternalInput")
    # pre-masked doubled positives, replicated 4x over the pair-of-block
    # index v with the v-mask baked in on the host:
    # posd4[r, v, 64h+p] = pred_pos[r,p] if (h==r%2 and (r%8)//2==v) else +50
    posd_d = nc.dram_tensor("posd", [RPC, 4 * 2 * K], BF16,
                            kind="ExternalInput")
    # col 0: P exp scale -(1+p%4); cols 8-11: one-hot coefficient masks
    # M1..M4 (p%4 == k)
    consts_d = nc.dram_tensor("consts", [128, 16], F32, kind="ExternalInput")
    y = nc.dram_tensor("y", [nreps, 128, NPASS], F32, kind="ExternalOutput")

    mul = mybir.AluOpType.mult
    add = mybir.AluOpType.add

    from contextlib import ExitStack
    with TileContext(nc) as tc, ExitStack() as st:
        io = st.enter_context(tc.tile_pool(name="io", bufs=1))
        ps = st.enter_context(tc.tile_pool(name="ps", bufs=1, space="PSUM"))

        # Trigger the exp/ln activation-table load ASAP (~1.3us on ScalarE,
        # overlapping the input DMA).
        d0 = io.tile([128, 1], F32, tag="d0")
        nc.vector.memset(d0, 0.0)
        d1 = io.tile([128, 1], BF16, tag="d1")
        nc.scalar.activation(d1, d0, EXP)

        # dummy operands for the PE warm-up chain
        dwr = io.tile([8, WARM_COLS], BF16, tag="dwr")
        nc.vector.memset(dwr, 0.0)

        for rep in range(nreps):
            # consts on the Pool SWDGE queue (parallel with SP inputs)
            cs = io.tile([128, 16], F32, tag="cs")
            nc.gpsimd.dma_start(out=cs, in_=consts_d[:])
            m1 = cs[:, 8:9]
            m2 = cs[:, 9:10]
            m3 = cs[:, 10:11]
            m4 = cs[:, 11:12]

            # inputs, replicated 4x across power slots (partition = 4r+k)
            # via stride-0 DRAM reads; negatives first (they gate the chain)
            negs = io.tile([128, NEG], BF16, tag="negs")
            nc.sync.dma_start(
                out=negs,
                in_=negs_d[:].unsqueeze(1).broadcast_to([RPC, 4, NEG]))
            posd = io.tile([128, 4 * 2 * K], BF16, tag="posd")
            nc.sync.dma_start(
                out=posd,
                in_=posd_d[:].unsqueeze(1).broadcast_to([RPC, 4, 4 * 2 * K]))

            # PE warm-up: keep the clock ramping from ~1us until the real
            # matmuls (psum scratch, no consumers)
            pw = ps.tile([128, WARM_COLS], F32, tag="pw")
            for w in range(N_WARM):
                nc.tensor.matmul(pw[0:WARM_COLS], dwr, dwr,
                                 start=True, stop=True)

            # ---- DVE early window (negs land long before exp finishes):
            # log-space sums so the product folds become ACT exps:
            #   m1 = E1*E2 = exp(n1+n2), m2 = E3*E4 = exp(n3+n4),
            #   v = m1*m2 = exp(n1+n2+n3+n4)
            # f32 sums keep exp() accurate (bf16 sums cost ~3% on exp) ----
            nsum = io.tile([128, 2 * NQ], F32, tag="nsum")
            nc.vector.tensor_tensor(nsum[:, 0:NQ], negs[:, 0:NQ],
                                    negs[:, NQ:2 * NQ], add)
            nc.vector.tensor_tensor(nsum[:, NQ:2 * NQ],
                                    negs[:, 2 * NQ:3 * NQ],
                                    negs[:, 3 * NQ:4 * NQ], add)

            # ---- ScalarE stream: exp chunks C1/C2 over negatives, product
            # folds M/V from the log sums, then the four pair-masked power
            # tiles P_v = exp(posd*scale_v + bias_v) ----
            e = io.tile([128, NEG], BF16, tag="e")
            nc.scalar.activation(e, negs, EXP)
            m = io.tile([128, 2 * NQ], BF16, tag="m")
            nc.scalar.activation(m, nsum, EXP)
            # ONE exp for all four pair-masked power tiles (v-mask baked in
            # the host data, power k in the per-partition scale): P_all's
            # column block v holds P_v
            p_all = io.tile([128, 4 * 2 * K], BF16, tag="Pall")
            nc.scalar.activation(p_all, posd, EXP, scale=cs[:, 0:1])
            pv = [p_all[:, 2 * K * vi:2 * K * (vi + 1)] for vi in range(4)]

            # ---- DVE: additive pair folds ----
            a = io.tile([128, 2 * NQ], BF16, tag="a")
            nc.vector.tensor_tensor(a[:, 0:NQ], e[:, 0:NQ], e[:, NQ:2 * NQ],
                                    add)
            nc.vector.tensor_tensor(a[:, NQ:2 * NQ], e[:, 2 * NQ:3 * NQ],
                                    e[:, 3 * NQ:4 * NQ], add)
            a1 = a[:, 0:NQ]
            a2 = a[:, NQ:2 * NQ]
            mm1 = m[:, 0:NQ]
            mm2 = m[:, NQ:2 * NQ]

            # ---- masked coefficient placement:
            # q[4r+k] = c_{k+1}[r]; Mk are per-partition 0/1 scalars ----
            w = io.tile([128, NQ], BF16, tag="w")
            v = io.tile([128, NQ], BF16, tag="v")
            # two products on Pool (off the DVE critical path); Pool only
            # supports plain tensor_tensor, masking happens in the DVE stt
            # chain below
            nc.gpsimd.tensor_tensor(v, mm1, mm2, mul)
            nc.gpsimd.tensor_tensor(w, a2, mm1, mul)

            aa = io.tile([128, NQ], BF16, tag="aa")
            s_a = io.tile([128, NQ], BF16, tag="sa")
            t_sa = io.tile([128, NQ], BF16, tag="tsa")
            t_am = io.tile([128, NQ], BF16, tag="tam")
            s_m = io.tile([128, NQ], BF16, tag="sm")
            s2 = io.tile([128, NQ], BF16, tag="s2")
            t_s2 = io.tile([128, NQ], BF16, tag="ts2")
            s3 = io.tile([128, NQ], BF16, tag="s3")
            t_v4 = io.tile([128, NQ], BF16, tag="tv4")
            z2 = io.tile([128, NQ], BF16, tag="z2")
            u1 = io.tile([128, NQ], BF16, tag="u1")
            q = io.tile([128, NQ], BF16, tag="q")
            # raw coefficients c2 = S_m + a1*a2 and c3 = a1*m2 + a2*m1 via
            # plain tensor ops; masks applied with 2x-mode tensor_scalar
            # (90ns) instead of scalar_tensor_tensor (177ns, no 2x)
            nc.vector.tensor_tensor(aa, a1, a2, mul)
            nc.vector.tensor_tensor(s_a, a1, a2, add)
            nc.vector.tensor_scalar(t_sa, s_a, m1, None, mul)
            nc.vector.tensor_tensor(t_am, a1, mm2, mul)
            nc.vector.tensor_tensor(s_m, mm1, mm2, add)
            nc.vector.tensor_tensor(s2, s_m, aa, add)
            nc.vector.tensor_scalar(t_s2, s2, m2, None, mul)
            nc.vector.tensor_tensor(s3, t_am, w, add)
            nc.vector.tensor_scalar(t_v4, v, m4, None, mul)
            nc.vector.scalar_tensor_tensor(z2, s3, m3, t_v4, mul, add)
            nc.vector.tensor_tensor(u1, t_sa, t_s2, add)
            nc.vector.tensor_tensor(q, u1, z2, add)

            # ---- matmuls: psum[64h+p, j] = sum_k F^{k+1}[2t+h, p] *
            # c_{k+1}[2t+h, j]; contraction is the aligned 32-block of 4
            # pairs, P_{t%4} zeroes the other 3.  psum: 4 pairs per 512-col
            # bank (a matmul output cannot cross a bank boundary).
            #
            # Matmul is linear in rhs, so pass-0 pairs {0,4,8,12} (all v=0,
            # needing only P0 which lands first) accumulate THREE early
            # matmul waves rhs = y1, y2, z — their PSUM is complete before
            # the q tile even exists, pulling the first Ln pass forward.
            # Remaining pairs use a single matmul on q = y3 + z. ----
            NB1 = (NPAIR - N0) // 4
            # each pass-0 pair gets its OWN bank (own 2KB zero region), so
            # the three accumulation waves form independent per-bank groups
            pt0 = ps.tile([128, N0 * 512], F32, tag="ps0")
            pt1 = ps.tile([128, NB1 * 512], F32, tag="ps1")
            p0_pairs = [0, 1, 2, 3]
            for wi, rhs_t in enumerate((t_sa, t_s2, z2)):
                for i, t in enumerate(p0_pairs):
                    blk = 32 * (t // 4)
                    nc.tensor.matmul(pt0[:, 512 * i:512 * i + NQ],
                                     pv[t % 4][blk:blk + 32, :],
                                     rhs_t[blk:blk + 32, :],
                                     start=(wi == 0), stop=(wi == 2),
                                     tile_position=(blk, 0))
                if wi < 2:
                    for w in range(N_FILL[wi]):
                        nc.tensor.matmul(pw[0:WARM_COLS], dwr, dwr,
                                         start=True, stop=True)
            for w in range(N_WARM2):
                nc.tensor.matmul(pw[0:WARM_COLS], dwr, dwr,
                                 start=True, stop=True)
            u = 0
            for t in range(NPAIR):
                if t in p0_pairs:
                    continue
                base = 512 * (u // 4) + NQ * (u % 4)
                blk = 32 * (t // 4)
                nc.tensor.matmul(pt1[:, base:base + NQ],
                                 pv[t % 4][blk:blk + 32, :],
                                 q[blk:blk + 32, :],
                                 start=True, stop=True,
                                 tile_position=(blk, 0))
                u += 1

            # ---- ln(1 + psum), accumulated per partition; asymmetric
            # passes so pass 0 starts after only N0 pairs of matmuls.
            # Throwaway ln outputs land in spare PSUM (access init 172 cyc
            # < SBUF 222). ----
            # Ln pass 0's throwaway output goes to SBUF; pass 1's overwrites
            # the then-dead pt0 banks (PSUM budget: N0+NB1 banks + warm = 8)
            partials = io.tile([128, NPASS], F32, tag="part")
            sout0 = io.tile([128, N0 * NQ], BF16, tag="sout0")
            nc.scalar.activation(
                sout0.rearrange("p (b x) -> p b x", x=NQ),
                pt0.rearrange("p (b x) -> p b x", x=512)[:, :, 0:NQ],
                LN, bias=1.0, accum_out=partials[:, 0:1])
            nc.scalar.activation(
                pt0[:, 0:(NPAIR - N0) * NQ].rearrange(
                    "p (b x) -> p b x", x=4 * NQ),
                pt1.rearrange("p (b x) -> p b x", x=512)[:, :, 0:4 * NQ],
                LN, bias=1.0, accum_out=partials[:, 1:2])

            nc.sync.dma_start(out=y[rep], in_=partials)

    nc.finalize()
    _PROG_CACHE[nreps] = (nc, ())
    return nc, ()


def _consts():
    cs = np.zeros((128, 16), dtype=np.float32)
    p = np.arange(128)
    k = p % 4
    cs[:, 0] = -(1.0 + k)
    for i in range(4):
        cs[:, 8 + i] = (k == i).astype(np.float32)
    return cs


def make_in_maps(prediction, target, consts):
    # fold validity into the prediction: invalid positives -> +50
    # (F^k = e^{-50k} -> 0), invalid negatives -> -50 (E = e^-50 -> 0)
    fill = np.empty((1, N), np.float32)
    fill[:, 0:K] = 50.0
    fill[:, K:N] = -50.0
    pred_m = np.where(target == -1, fill, prediction).astype(np.float32)
    csv = _consts()
    in_maps = []
    for c in range(NC):
        blk = pred_m[c * RPC:(c + 1) * RPC]
        negs = np.ascontiguousarray(blk[:, K:N]).astype(ml_dtypes.bfloat16)
        # doubled positives with the pair-interleave AND pair-of-block
        # masks folded in: posd4[r, v, 64h:64h+64] = pos row r if
        # (h == r%2 and (r%8)//2 == v) else +50
        posd4 = np.full((RPC, 4, 2, K), 50.0, np.float32)
        rr = np.arange(RPC)
        posd4[rr, (rr % 8) // 2, rr % 2, :] = blk[:, 0:K]
        in_maps.append({
            "negs": negs,
            "posd": posd4.reshape(RPC, 4 * 2 * K).astype(ml_dtypes.bfloat16),
            "consts": csv,
        })
    return in_maps


def kernel(prediction, target):
    nc, consts = build_program(1)
    in_maps = make_in_maps(prediction, target, consts)
    res = run_bass_kernel_spmd(nc, in_maps, core_ids=list(range(NC)))
    total = sum(float(res.results[c]["y"][0].sum(dtype=np.float64))
                for c in range(NC))
    return np.float32(total / B)


# revision 33
# speedup vs baseline: 1.0351x; 1.0351x over previous
"""BPR pairwise softplus loss on 8 Trainium2 NeuronCores.

loss = mean_b sum_{i<K, j>=K, both valid} softplus(pred[b,j] - pred[b,i])

Strategy (data parallel over batch, 32 rows/core), folding FOUR negatives
per ln via elementary symmetric polynomials:

  prod_{m=1..4} (1 + F*E_m) = 1 + F*c1 + F^2*c2 + F^3*c3 + F^4*c4
  =>  sum_m softplus(n_m - p) = ln(1 + sum_k F^k c_k),  F = exp(-p), E = exp(n)

Pack-free layout: partition = 4*r + k (row-major, power k innermost), so the
8 contraction partitions of row pair t = (2t, 2t+1) are the contiguous range
8t..8t+8 and the matmul reads the power tile P and coefficient tile q
DIRECTLY - no SBUF->SBUF pack DMAs (each DMA hop costs ~2.7us of fixed
latency: 565 SEQ + 625 HWDGE + 650 engine delay + transfer + 900 sem).

  - P_v[4r+k, 64h+p] = F^{k+1}[r, p] masked to half h == r%2 AND to pairs
    with (r//2)%4 == v: each P_v is ONE ScalarE exp with per-partition
    scale -(k+1) / bias 0 on live partitions and scale 0 / bias -100
    (exp -> 0) elsewhere; the column-interleave mask is folded into the
    host data (+50 fill -> exp(-(k+1)*50) = 0).
  - q[4r+k, j] = c_{k+1}[r, j]: pair folds a/m then masked placement with
    per-partition 0/1 scalar masks via scalar_tensor_tensor:
      q = M1*(a1+a2) + M2*(m1+m2+a1*a2) + M3*(a1*m2+a2*m1) + M4*(m1*m2)
  - 16 matmuls (contraction 32 = one aligned block of 4 pairs, lhsT P_v
    zeroing the other 3 pairs, free 112) straight into PSUM; asymmetric Ln
    passes (N0 pairs then 16-N0) with accum_out row sums; one output DMA.
    (PE tiling: operands must sit at partition base 0/32/64/96 with an
    explicit tile_position, hence the 32-block contraction.)
  - Dummy warm matmuls from ~1us keep the PE p-state ramping so the real
    matmuls run at full clock (a PE idle gap resets the 3us ramp).

Invalid slots (target == -1) fold into the prediction on the host: invalid
positives -> +50 (F^k -> 0), invalid negatives -> -50 (E -> 0).
"""
import sys

sys.path.insert(0, "/opt/trn_rl_repo")

import numpy as np
import ml_dtypes

import concourse.bass as bass
import concourse.mybir as mybir
from concourse import bacc
import concourse.hw_specs as hw_specs
from concourse.tile import TileContext
from concourse.bass_utils import run_bass_kernel_spmd

B, N, K = 256, 512, 64
NC = 8
RPC = B // NC            # 32 batch rows per core
NPAIR = RPC // 2         # 16 row pairs (2t, 2t+1)
NEG = N - K              # 448 negatives per row
G = 4                    # negatives folded per ln
NQ = NEG // G            # 112 quad groups per row
NPASS = 2                # Ln passes
N0 = 4                   # row pairs in Ln pass 0 (rest in pass 1); multiple
                         # of 4 so each pass covers whole PSUM banks
N_WARM = 336             # PE clock warm-up dummy matmuls
N_WARM2 = 0             # filler matmuls between the last wave and the q
                         # matmuls so PE never idles (idle resets p-state)
N_FILL = (0, 0)        # fillers after waves 1 and 2
WARM_COLS = 16           # free size of each warm matmul

_PROG_CACHE = {}

EXP = mybir.ActivationFunctionType.Exp
LN = mybir.ActivationFunctionType.Ln
F32 = mybir.dt.float32
BF16 = mybir.dt.bfloat16


def _patch_act_tables():
    """Make natural_log_exp_and_others the only table set advertising exp/ln
    so Bacc's table-load pass emits a single ACT_TABLE_LOAD."""
    if getattr(hw_specs.get_activation_tables, "_bpr_patched", False):
        return
    orig_fn = hw_specs.get_activation_tables

    def patched(arch):
        d = orig_fn(arch)
        out = {}
        for name, funcs in d.items():
            if name != "natural_log_exp_and_others" and (EXP in funcs
                                                         or LN in funcs):
                funcs = funcs - {EXP, LN}
            out[name] = funcs
        return out

    patched._bpr_patched = True
    hw_specs.get_activation_tables = patched
    bacc.get_activation_tables = patched

    # Bass.__init__ materializes 4 const APs via gpsimd.memset, serializing
    # ~440ns on Pool before the start barrier that gates the first input
    # DMA.  Spread them DVE/Pool round-robin to halve that chain.
    orig_memset = bass.BassEitherVectorEngine.memset

    def memset_rr(self, ap, constant):
        name = getattr(getattr(ap, "tensor", None), "name", "") or ""
        if name in ("const-bfloat16-1.0", "const-uint8-127"):
            return None  # dead consts (no readers in this program)
        if name.startswith("const-"):
            b = self.bass
            rr = getattr(b, "_bpr_const_rr", 0)
            b._bpr_const_rr = rr + 1
            eng = b.vector if rr % 2 == 0 else b.gpsimd
            if eng is not self:
                return orig_memset(eng, ap, constant)
        return orig_memset(self, ap, constant)

    bass.BassEitherVectorEngine.memset = memset_rr

    # Skip the all-engine barrier Bass.__init__ emits right after the const
    # memsets — it serializes the first input DMA behind them, but nothing
    # the DMA queue touches depends on the consts, and the first const
    # consumer (an ACT op at ~1.7us) runs >1.5us after the memsets land.
    orig_barrier = bass.Bass.all_engine_barrier

    def barrier_skip_init(self, *, sem_only=False):
        if not hasattr(self, "_init_sbuf_base"):
            return None  # the one call during __init__
        return orig_barrier(self, sem_only=sem_only)

    bass.Bass.all_engine_barrier = barrier_skip_init


def build_program(nreps: int = 1):
    if nreps in _PROG_CACHE:
        return _PROG_CACHE[nreps]
    _patch_act_tables()
    nc = bacc.Bacc("TRN2", target_bir_lowering=False, debug=False,
                   num_devices=NC)
    negs_d = nc.dram_tensor("negs", [RPC, NEG], BF16, kind="ExternalInput")
    # pre-masked doubled positives, replicated 4x over the pair-of-block
    # index v with the v-mask baked in on the host:
    # posd4[r, v, 64h+p] = pred_pos[r,p] if (h==r%2 and (r%8)//2==v) else +50
    posd_d = nc.dram_tensor("posd", [RPC, 4 * 2 * K], BF16,
                            kind="ExternalInput")
    # col 0: P exp scale -(1+p%4); cols 8-11: one-hot coefficient masks
    # M1..M4 (p%4 == k)
    consts_d = nc.dram_tensor("consts", [128, 16], F32, kind="ExternalInput")
    y = nc.dram_tensor("y", [nreps, 128, NPASS], F32, kind="ExternalOutput")

    mul = mybir.AluOpType.mult
    add = mybir.AluOpType.add

    from contextlib import ExitStack
    with TileContext(nc) as tc, ExitStack() as st:
        io = st.enter_context(tc.tile_pool(name="io", bufs=1))
        ps = st.enter_context(tc.tile_pool(name="ps", bufs=1, space="PSUM"))

        # Trigger the exp/ln activation-table load ASAP (~1.3us on ScalarE,
        # overlapping the input DMA).
        d0 = io.tile([128, 1], F32, tag="d0")
        nc.vector.memset(d0, 0.0)
        d1 = io.tile([128, 1], BF16, tag="d1")
        nc.scalar.activation(d1, d0, EXP)

        # dummy operands for the PE warm-up chain
        dwr = io.tile([8, WARM_COLS], BF16, tag="dwr")
        nc.vector.memset(dwr, 0.0)

        for rep in range(nreps):
            # consts on the Pool SWDGE queue (parallel with SP inputs)
            cs = io.tile([128, 16], F32, tag="cs")
            nc.gpsimd.dma_start(out=cs, in_=consts_d[:])
            m1 = cs[:, 8:9]
            m2 = cs[:, 9:10]
            m3 = cs[:, 10:11]
            m4 = cs[:, 11:12]

            # inputs, replicated 4x across power slots (partition = 4r+k)
            # via stride-0 DRAM reads; negatives first (they gate the chain)
            negs = io.tile([128, NEG], BF16, tag="negs")
            nc.sync.dma_start(
                out=negs,
                in_=negs_d[:].unsqueeze(1).broadcast_to([RPC, 4, NEG]))
            posd = io.tile([128, 4 * 2 * K], BF16, tag="posd")
            nc.sync.dma_start(
                out=posd,
                in_=posd_d[:].unsqueeze(1).broadcast_to([RPC, 4, 4 * 2 * K]))

            # PE warm-up: keep the clock ramping from ~1us until the real
            # matmuls (psum scratch, no consumers)
            pw = ps.tile([128, WARM_COLS], F32, tag="pw")
            for w in range(N_WARM):
                nc.tensor.matmul(pw[0:WARM_COLS], dwr, dwr,
                                 start=True, stop=True)

            # ---- DVE early window (negs land long before exp finishes):
            # log-space sums so the product folds become ACT exps:
            #   m1 = E1*E2 = exp(n1+n2), m2 = E3*E4 = exp(n3+n4),
            #   v = m1*m2 = exp(n1+n2+n3+n4)
            # f32 sums keep exp() accurate (bf16 sums cost ~3% on exp) ----
            nsum = io.tile([128, 2 * NQ], F32, tag="nsum")
            nc.vector.tensor_tensor(nsum[:, 0:NQ], negs[:, 0:NQ],
                                    negs[:, NQ:2 * NQ], add)
            nc.vector.tensor_tensor(nsum[:, NQ:2 * NQ],
                                    negs[:, 2 * NQ:3 * NQ],
                                    negs[:, 3 * NQ:4 * NQ], add)

            # ---- ScalarE stream: exp chunks C1/C2 over negatives, product
            # folds M/V from the log sums, then the four pair-masked power
            # tiles P_v = exp(posd*scale_v + bias_v) ----
            e = io.tile([128, NEG], BF16, tag="e")
            nc.scalar.activation(e, negs, EXP)
            m = io.tile([128, 2 * NQ], BF16, tag="m")
            nc.scalar.activation(m, nsum, EXP)
            # ONE exp for all four pair-masked power tiles (v-mask baked in
            # the host data, power k in the per-partition scale): P_all's
            # column block v holds P_v
            p_all = io.tile([128, 4 * 2 * K], BF16, tag="Pall")
            nc.scalar.activation(p_all, posd, EXP, scale=cs[:, 0:1])
            pv = [p_all[:, 2 * K * vi:2 * K * (vi + 1)] for vi in range(4)]

            # ---- DVE: additive pair folds ----
            a = io.tile([128, 2 * NQ], BF16, tag="a")
            nc.vector.tensor_tensor(a[:, 0:NQ], e[:, 0:NQ], e[:, NQ:2 * NQ],
                                    add)
            nc.vector.tensor_tensor(a[:, NQ:2 * NQ], e[:, 2 * NQ:3 * NQ],
                                    e[:, 3 * NQ:4 * NQ], add)
            a1 = a[:, 0:NQ]
            a2 = a[:, NQ:2 * NQ]
            mm1 = m[:, 0:NQ]
            mm2 = m[:, NQ:2 * NQ]

            # ---- masked coefficient placement:
            # q[4r+k] = c_{k+1}[r]; Mk are per-partition 0/1 scalars ----
            w = io.tile([128, NQ], BF16, tag="w")
            v = io.tile([128, NQ], BF16, tag="v")
            # two products on Pool (off the DVE critical path); Pool only
            # supports plain tensor_tensor, masking happens in the DVE stt
            # chain below
            nc.gpsimd.tensor_tensor(v, mm1, mm2, mul)
            nc.gpsimd.tensor_tensor(w, a2, mm1, mul)

            aa = io.tile([128, NQ], BF16, tag="aa")
            s_a = io.tile([128, NQ], BF16, tag="sa")
            t_sa = io.tile([128, NQ], BF16, tag="tsa")
            t_am = io.tile([128, NQ], BF16, tag="tam")
            s_m = io.tile([128, NQ], BF16, tag="sm")
            s2 = io.tile([128, NQ], BF16, tag="s2")
            t_s2 = io.tile([128, NQ], BF16, tag="ts2")
            s3 = io.tile([128, NQ], BF16, tag="s3")
            t_v4 = io.tile([128, NQ], BF16, tag="tv4")
            z2 = io.tile([128, NQ], BF16, tag="z2")
            u1 = io.tile([128, NQ], BF16, tag="u1")
            q = io.tile([128, NQ], BF16, tag="q")
            # raw coefficients c2 = S_m + a1*a2 and c3 = a1*m2 + a2*m1 via
            # plain tensor ops; masks applied with 2x-mode tensor_scalar
            # (90ns) instead of scalar_tensor_tensor (177ns, no 2x)
            nc.vector.tensor_tensor(aa, a1, a2, mul)
            nc.vector.tensor_tensor(s_a, a1, a2, add)
            nc.vector.tensor_tensor(t_am, a1, mm2, mul)
            nc.vector.tensor_tensor(s_m, mm1, mm2, add)
            nc.vector.tensor_scalar(t_sa, s_a, m1, None, mul)
            nc.vector.tensor_tensor(s2, s_m, aa, add)
            nc.vector.tensor_scalar(t_v4, v, m4, None, mul)
            nc.vector.tensor_tensor(s3, t_am, w, add)
            nc.vector.tensor_scalar(t_s2, s2, m2, None, mul)
            nc.vector.scalar_tensor_tensor(z2, s3, m3, t_v4, mul, add)
            nc.vector.tensor_tensor(u1, t_sa, t_s2, add)
            nc.vector.tensor_tensor(q, u1, z2, add)

            # ---- matmuls: psum[64h+p, j] = sum_k F^{k+1}[2t+h, p] *
            # c_{k+1}[2t+h, j]; contraction is the aligned 32-block of 4
            # pairs, P_{t%4} zeroes the other 3.  psum: 4 pairs per 512-col
            # bank (a matmul output cannot cross a bank boundary).
            #
            # Matmul is linear in rhs, so pass-0 pairs {0,4,8,12} (all v=0,
            # needing only P0 which lands first) accumulate THREE early
            # matmul waves rhs = y1, y2, z — their PSUM is complete before
            # the q tile even exists, pulling the first Ln pass forward.
            # Remaining pairs use a single matmul on q = y3 + z. ----
            NB1 = (NPAIR - N0) // 4
            # each pass-0 pair gets its OWN bank (own 2KB zero region), so
            # the three accumulation waves form independent per-bank groups
            pt0 = ps.tile([128, N0 * 512], F32, tag="ps0")
            pt1 = ps.tile([128, NB1 * 512], F32, tag="ps1")
            p0_pairs = [0, 1, 2, 3]
            for wi, rhs_t in enumerate((t_sa, t_s2, z2)):
                for i, t in enumerate(p0_pairs):
                    blk = 32 * (t // 4)
                    nc.tensor.matmul(pt0[:, 512 * i:512 * i + NQ],
                                     pv[t % 4][blk:blk + 32, :],
                                     rhs_t[blk:blk + 32, :],
                                     start=(wi == 0), stop=(wi == 2),
                                     tile_position=(blk, 0))
                if wi < 2:
                    for w in range(N_FILL[wi]):
                        nc.tensor.matmul(pw[0:WARM_COLS], dwr, dwr,
                                         start=True, stop=True)
            for w in range(N_WARM2):
                nc.tensor.matmul(pw[0:WARM_COLS], dwr, dwr,
                                 start=True, stop=True)
            u = 0
            for t in range(NPAIR):
                if t in p0_pairs:
                    continue
                base = 512 * (u // 4) + NQ * (u % 4)
                blk = 32 * (t // 4)
                nc.tensor.matmul(pt1[:, base:base + NQ],
                                 pv[t % 4][blk:blk + 32, :],
                                 q[blk:blk + 32, :],
                                 start=True, stop=True,
                                 tile_position=(blk, 0))
                u += 1

            # ---- ln(1 + psum), accumulated per partition; asymmetric
            # passes so pass 0 starts after only N0 pairs of matmuls.
            # Throwaway ln outputs land in spare PSUM (access init 172 cyc
            # < SBUF 222). ----
            # Ln pass 0's throwaway output goes to SBUF; pass 1's overwrites
            # the then-dead pt0 banks (PSUM budget: N0+NB1 banks + warm = 8)
            partials = io.tile([128, NPASS], F32, tag="part")
            sout0 = io.tile([128, N0 * NQ], BF16, tag="sout0")
            nc.scalar.activation(
                sout0.rearrange("p (b x) -> p b x", x=NQ),
                pt0.rearrange("p (b x) -> p b x", x=512)[:, :, 0:NQ],
                LN, bias=1.0, accum_out=partials[:, 0:1])
            nc.scalar.activation(
                pt0[:, 0:(NPAIR - N0) * NQ].rearrange(
                    "p (b x) -> p b x", x=4 * NQ),
                pt1.rearrange("p (b x) -> p b x", x=512)[:, :, 0:4 * NQ],
                LN, bias=1.0, accum_out=partials[:, 1:2])

            nc.sync.dma_start(out=y[rep], in_=partials)

    nc.finalize()
    _PROG_CACHE[nreps] = (nc, ())
    return nc, ()


def _consts():
    cs = np.zeros((128, 16), dtype=np.float32)
    p = np.arange(128)
    k = p % 4
    cs[:, 0] = -(1.0 + k)
    for i in range(4):
        cs[:, 8 + i] = (k == i).astype(np.float32)
    return cs


def make_in_maps(prediction, target, consts):
    # fold validity into the prediction: invalid positives -> +50
    # (F^k = e^{-50k} -> 0), invalid negatives -> -50 (E = e^-50 -> 0)
    fill = np.empty((1, N), np.float32)
    fill[:, 0:K] = 50.0
    fill[:, K:N] = -50.0
    pred_m = np.where(target == -1, fill, prediction).astype(np.float32)
    csv = _consts()
    in_maps = []
    for c in range(NC):
        blk = pred_m[c * RPC:(c + 1) * RPC]
        negs = np.ascontiguousarray(blk[:, K:N]).astype(ml_dtypes.bfloat16)
        # doubled positives with the pair-interleave AND pair-of-block
        # masks folded in: posd4[r, v, 64h:64h+64] = pos row r if
        # (h == r%2 and (r%8)//2 == v) else +50
        posd4 = np.full((RPC, 4, 2, K), 50.0, np.float32)
        rr = np.arange(RPC)
        posd4[rr, (rr % 8) // 2, rr % 2, :] = blk[:, 0:K]
        in_maps.append({
            "negs": negs,
            "posd": posd4.reshape(RPC, 4 * 2 * K).astype(ml_dtypes.bfloat16),
            "consts": csv,
        })
    return in_maps


def kernel(prediction, target):
    nc, consts = build_program(1)
    in_maps = make_in_maps(prediction, target, consts)
    res = run_bass_kernel_spmd(nc, in_maps, core_ids=list(range(NC)))
    total = sum(float(res.results[c]["y"][0].sum(dtype=np.float64))
                for c in range(NC))
    return np.float32(total / B)
